# revision 2
# baseline (speedup 1.0000x reference)
"""Trainium2 kernel v3 for nn_CMSBlockLinear — mixed bf16 + fp8-DoubleRow dense
matmul, token-sharded 8 ways.

Strategy: densify the 50%-dense 16x16-block weights host-side and run a dense
[1024,2048]x[2048,8192] matmul per core. Contraction is split 1792 (bf16,
1.0 cyc/row) + 256 (fp8 e4m3 DoubleRow, 0.5 cyc/row effective) — the fp8
slice carries scales x*2^-3 / W*2^3 so its psum contribution needs no
rescale and chains into the same accumulation group as the bf16 matmuls.
Measured rel err ~1.33e-2 vs the 2e-2 gate.
"""

import os
import sys

sys.path.insert(0, "/opt/trn_rl_repo")

import numpy as np
import ml_dtypes

T, IN_F, OUT_F = 8192, 2048, 8192
NCORES = 8
TPC = T // NCORES  # 1024 tokens per core
KB = 14  # bf16 contraction chunks of 128 (k = 0..1791)
KF = IN_F - KB * 128  # 256 fp8 contraction tail
NT = OUT_F // 512  # 16 feature tiles of 512
MT = TPC // 8 // 16  # 8 token tiles of 128

GROUP = int(os.environ.get("K3_GROUP", "8"))
LASTGROUP = int(os.environ.get("K3_LASTGROUP", "2"))
NWARM = int(os.environ.get("K3_NWARM", "10"))
WBUFS = int(os.environ.get("K3_WBUFS", "32"))

_cached_nc = None


def _build_program():
    global _cached_nc
    if _cached_nc is not None:
        return _cached_nc
    from concourse import bacc, mybir, tile

    F32, BF16, FP8 = mybir.dt.float32, mybir.dt.bfloat16, mybir.dt.float8e4
    DRMODE = mybir.MatmulPerfMode.DoubleRow

    nc = bacc.Bacc(None)
    xT = nc.declare_dram_parameter("xT", [128, KB, TPC], BF16, isOutput=False)
    x8 = nc.declare_dram_parameter("x8", [128, MT, 2, 128], FP8, isOutput=False)
    W = nc.declare_dram_parameter("W", [NT, KB, 128, 512], BF16, isOutput=False)
    W8 = nc.declare_dram_parameter("W8", [NT, 2, 128, 2, 256], FP8, isOutput=False)
    out = nc.declare_dram_parameter("out", [TPC, OUT_F], F32, isOutput=True)

    with tile.TileContext(nc) as tc:
        with tc.tile_pool(name="xt", bufs=1) as xpool, \
             tc.tile_pool(name="wt", bufs=WBUFS) as wpool, \
             tc.tile_pool(name="w8t", bufs=8) as w8pool, \
             tc.tile_pool(name="ot", bufs=12) as opool, \
             tc.tile_pool(name="ps", bufs=1, space="PSUM") as ps:
            xts = []
            for ko in range(KB):
                xk = xpool.tile([128, TPC], BF16, tag=f"x{ko}", name=f"xk{ko}")
                nc.gpsimd.dma_start(out=xk[:], in_=xT[:, ko, :])
                xts.append(xk)
            x8ms = []
            for m in range(MT):
                x8m = xpool.tile([128, 2, 128], FP8, tag=f"x8_{m}", name=f"x8_{m}")
                nc.gpsimd.dma_start(out=x8m[:], in_=x8[:, m])
                x8ms.append(x8m)
            # pstate ramp on the first x tile as soon as its DMA lands
            # (~8us); values are irrelevant (psum slot p7 is overwritten by
            # start=True later).
            wps = ps.tile([128, 512], F32, tag=f"p{MT-1}", name="warm_ps")
            for i in range(NWARM):
                nc.tensor.matmul(
                    wps[:], xts[0][:, :128], xts[0][:, :512],
                    start=True, stop=True,
                )
            for n in range(NT):
                psums = [
                    ps.tile([128, 512], F32, tag=f"p{m}", name=f"ps{n}_{m}")
                    for m in range(MT)
                ]
                wts = []
                for ko in range(KB):
                    wt = wpool.tile([128, 512], BF16, tag="w", name=f"w{n}_{ko}")
                    nc.sync.dma_start(out=wt[:], in_=W[n, ko])
                    wts.append(wt)
                w8s = []
                for h in range(2):
                    w8tile = w8pool.tile(
                        [128, 2, 256], FP8, tag="w8", name=f"w8_{n}_{h}"
                    )
                    nc.sync.dma_start(out=w8tile[:], in_=W8[n, h])
                    w8s.append(w8tile)
                grp = GROUP if n < NT - 1 else LASTGROUP
                for mg in range(0, MT, grp):
                    ms = range(mg, mg + grp)
                    for ko in range(KB):
                        for m in ms:
                            nc.tensor.matmul(
                                psums[m][:],
                                xts[ko][:, m * 128 : (m + 1) * 128],
                                wts[ko][:],
                                start=(ko == 0),
                                stop=False,
                            )
                    for m in ms:
                        for h in range(2):
                            nc.tensor.matmul(
                                psums[m][:, h * 256 : (h + 1) * 256],
                                x8ms[m][:],
                                w8s[h][:],
                                start=False,
                                stop=True,
                                perf_mode=DRMODE,
                            )
                    for m in ms:
                        ot = opool.tile([128, 512], F32, tag="o", name=f"o{n}_{m}")
                        nc.vector.tensor_copy(ot[:], psums[m][:])
                        nc.scalar.dma_start(
                            out=out[m * 128 : (m + 1) * 128, n * 512 : (n + 1) * 512],
                            in_=ot[:],
                        )
    nc.compile()
    _cached_nc = nc
    return nc


def _prep_inputs(x, values, bias, col_indices):
    x = np.ascontiguousarray(np.asarray(x), dtype=np.float32)
    values = np.ascontiguousarray(np.asarray(values), dtype=np.float32)
    bias = np.asarray(bias, dtype=np.float32)
    col_indices = np.asarray(col_indices, dtype=np.int32)

    R, K = col_indices.shape  # 512, 64
    C = IN_F // 16  # 128 column blocks

    Wb = np.zeros((C, R, 16, 16), np.float32)  # [c, r, i, o]
    r_idx = np.broadcast_to(np.arange(R, dtype=np.int64)[:, None], col_indices.shape)
    Wb[col_indices, r_idx] = values.transpose(0, 1, 3, 2)  # values[r,k,o,i] -> [i,o]
    Wd = Wb.transpose(0, 2, 1, 3).reshape(IN_F, OUT_F)

    KBF = KB * 128  # 1792
    Wb16 = Wd[:KBF].astype(ml_dtypes.bfloat16)
    W4 = np.ascontiguousarray(
        Wb16.reshape(KB, 128, NT, 512).transpose(2, 0, 1, 3)
    )  # [NT, KB, 128, 512]
    # fp8 tail: W8[n, h, p, i, j] = Wd[KBF + i*128 + p, n*512 + h*256 + j] * 8
    Wtail = (Wd[KBF:] * 8.0).astype(ml_dtypes.float8_e4m3)  # [256, OUT_F]
    W8 = np.ascontiguousarray(
        Wtail.reshape(2, 128, NT, 2, 256).transpose(2, 3, 1, 0, 4)
    )  # [NT, 2, 128, 2, 256]

    in_maps = []
    for c in range(NCORES):
        xs = x[c * TPC : (c + 1) * TPC]  # [TPC, IN_F]
        xTc = np.ascontiguousarray(
            xs[:, :KBF].T.reshape(KB, 128, TPC).transpose(1, 0, 2)
        ).astype(ml_dtypes.bfloat16)  # [128, KB, TPC]
        # x8[p, m, i, t] = xs[m*128 + t, KBF + i*128 + p] / 8
        xt8 = (xs[:, KBF:] * 0.125).astype(ml_dtypes.float8_e4m3)  # [TPC, 256]
        x8c = np.ascontiguousarray(
            xt8.reshape(MT, 128, 2, 128).transpose(3, 0, 2, 1)
        )  # [128, MT, 2, 128]
        in_maps.append({"xT": xTc, "x8": x8c, "W": W4, "W8": W8})
    return in_maps, bias


def _run(x, values, bias, col_indices, trace=False):
    from concourse.bass_utils import run_bass_kernel_spmd

    nc = _build_program()
    in_maps, bias_np = _prep_inputs(x, values, bias, col_indices)
    kwargs = {}
    if trace:
        import tempfile

        kwargs["tmpdir"] = tempfile.mkdtemp(prefix="bass_trace_")
    try:
        res = run_bass_kernel_spmd(
            nc, in_maps, list(range(NCORES)), trace=trace, **kwargs
        )
    except Exception:
        import time

        time.sleep(20)
        res = run_bass_kernel_spmd(
            nc, in_maps, list(range(NCORES)), trace=trace, **kwargs
        )
    out = np.concatenate([res.results[c]["out"] for c in range(NCORES)], axis=0)
    if np.any(bias_np):
        out = out + bias_np[None, :]
    return out, res


def kernel(x, values, bias, col_indices):
    out, _ = _run(x, values, bias, col_indices)
    return out


# revision 3
# speedup vs baseline: 1.0002x; 1.0002x over previous
"""Trainium2 kernel v3 for nn_CMSBlockLinear — mixed bf16 + fp8-DoubleRow dense
matmul, token-sharded 8 ways.

Strategy: densify the 50%-dense 16x16-block weights host-side and run a dense
[1024,2048]x[2048,8192] matmul per core. Contraction is split 1792 (bf16,
1.0 cyc/row) + 256 (fp8 e4m3 DoubleRow, 0.5 cyc/row effective) — the fp8
slice carries scales x*2^-3 / W*2^3 so its psum contribution needs no
rescale and chains into the same accumulation group as the bf16 matmuls.
Measured rel err ~1.33e-2 vs the 2e-2 gate.
"""

import os
import sys

sys.path.insert(0, "/opt/trn_rl_repo")

import numpy as np
import ml_dtypes

T, IN_F, OUT_F = 8192, 2048, 8192
NCORES = 8
TPC = T // NCORES  # 1024 tokens per core
KB = 14  # bf16 contraction chunks of 128 (k = 0..1791)
KF = IN_F - KB * 128  # 256 fp8 contraction tail
NT = OUT_F // 512  # 16 feature tiles of 512
MT = TPC // 8 // 16  # 8 token tiles of 128

GROUP = int(os.environ.get("K3_GROUP", "8"))
LASTGROUP = int(os.environ.get("K3_LASTGROUP", "1"))
NWARM = int(os.environ.get("K3_NWARM", "8"))
WBUFS = int(os.environ.get("K3_WBUFS", "32"))

_cached_nc = None


def _build_program():
    global _cached_nc
    if _cached_nc is not None:
        return _cached_nc
    from concourse import bacc, mybir, tile

    F32, BF16, FP8 = mybir.dt.float32, mybir.dt.bfloat16, mybir.dt.float8e4
    DRMODE = mybir.MatmulPerfMode.DoubleRow

    nc = bacc.Bacc(None)
    xT = nc.declare_dram_parameter("xT", [128, KB, TPC], BF16, isOutput=False)
    x8 = nc.declare_dram_parameter("x8", [128, MT, 2, 128], FP8, isOutput=False)
    W = nc.declare_dram_parameter("W", [NT, KB, 128, 512], BF16, isOutput=False)
    W8 = nc.declare_dram_parameter("W8", [NT, 2, 128, 2, 256], FP8, isOutput=False)
    out = nc.declare_dram_parameter("out", [TPC, OUT_F], F32, isOutput=True)

    with tile.TileContext(nc) as tc:
        with tc.tile_pool(name="xt", bufs=1) as xpool, \
             tc.tile_pool(name="wt", bufs=WBUFS) as wpool, \
             tc.tile_pool(name="w8t", bufs=8) as w8pool, \
             tc.tile_pool(name="ot", bufs=12) as opool, \
             tc.tile_pool(name="ps", bufs=1, space="PSUM") as ps:
            xts = []
            for ko in range(KB):
                xk = xpool.tile([128, TPC], BF16, tag=f"x{ko}", name=f"xk{ko}")
                # ko=0 rides the sync queue ahead of the W stream: it lands
                # ~6us earlier than gpsimd's first DMA, unblocking the
                # warmup (pstate ramp) sooner. Delays W[0,0] by only ~700ns.
                (nc.sync if ko == 0 else nc.gpsimd).dma_start(
                    out=xk[:], in_=xT[:, ko, :]
                )
                xts.append(xk)
            x8ms = []
            for m in range(MT):
                x8m = xpool.tile([128, 2, 128], FP8, tag=f"x8_{m}", name=f"x8_{m}")
                nc.gpsimd.dma_start(out=x8m[:], in_=x8[:, m])
                x8ms.append(x8m)
            # pstate ramp on the first x tile as soon as its DMA lands
            # (~8us); values are irrelevant (psum slot p7 is overwritten by
            # start=True later).
            wps = ps.tile([128, 512], F32, tag=f"p{MT-1}", name="warm_ps")
            for i in range(NWARM):
                nc.tensor.matmul(
                    wps[:], xts[0][:, :128], xts[0][:, :512],
                    start=True, stop=True,
                )
            for n in range(NT):
                psums = [
                    ps.tile([128, 512], F32, tag=f"p{m}", name=f"ps{n}_{m}")
                    for m in range(MT)
                ]
                wts = []
                for ko in range(KB):
                    wt = wpool.tile([128, 512], BF16, tag="w", name=f"w{n}_{ko}")
                    nc.sync.dma_start(out=wt[:], in_=W[n, ko])
                    wts.append(wt)
                w8s = []
                for h in range(2):
                    w8tile = w8pool.tile(
                        [128, 2, 256], FP8, tag="w8", name=f"w8_{n}_{h}"
                    )
                    nc.sync.dma_start(out=w8tile[:], in_=W8[n, h])
                    w8s.append(w8tile)
                grp = GROUP if n < NT - 1 else LASTGROUP
                for mg in range(0, MT, grp):
                    ms = range(mg, mg + grp)
                    for ko in range(KB):
                        for m in ms:
                            nc.tensor.matmul(
                                psums[m][:],
                                xts[ko][:, m * 128 : (m + 1) * 128],
                                wts[ko][:],
                                start=(ko == 0),
                                stop=False,
                            )
                    for m in ms:
                        for h in range(2):
                            nc.tensor.matmul(
                                psums[m][:, h * 256 : (h + 1) * 256],
                                x8ms[m][:],
                                w8s[h][:],
                                start=False,
                                stop=True,
                                perf_mode=DRMODE,
                            )
                    for m in ms:
                        ot = opool.tile([128, 512], F32, tag="o", name=f"o{n}_{m}")
                        nc.vector.tensor_copy(ot[:], psums[m][:])
                        nc.scalar.dma_start(
                            out=out[m * 128 : (m + 1) * 128, n * 512 : (n + 1) * 512],
                            in_=ot[:],
                        )
    nc.compile()
    _cached_nc = nc
    return nc


def _prep_inputs(x, values, bias, col_indices):
    x = np.ascontiguousarray(np.asarray(x), dtype=np.float32)
    values = np.ascontiguousarray(np.asarray(values), dtype=np.float32)
    bias = np.asarray(bias, dtype=np.float32)
    col_indices = np.asarray(col_indices, dtype=np.int32)

    R, K = col_indices.shape  # 512, 64
    C = IN_F // 16  # 128 column blocks

    Wb = np.zeros((C, R, 16, 16), np.float32)  # [c, r, i, o]
    r_idx = np.broadcast_to(np.arange(R, dtype=np.int64)[:, None], col_indices.shape)
    Wb[col_indices, r_idx] = values.transpose(0, 1, 3, 2)  # values[r,k,o,i] -> [i,o]
    Wd = Wb.transpose(0, 2, 1, 3).reshape(IN_F, OUT_F)

    KBF = KB * 128  # 1792
    Wb16 = Wd[:KBF].astype(ml_dtypes.bfloat16)
    W4 = np.ascontiguousarray(
        Wb16.reshape(KB, 128, NT, 512).transpose(2, 0, 1, 3)
    )  # [NT, KB, 128, 512]
    # fp8 tail: W8[n, h, p, i, j] = Wd[KBF + i*128 + p, n*512 + h*256 + j] * 8
    Wtail = (Wd[KBF:] * 8.0).astype(ml_dtypes.float8_e4m3)  # [256, OUT_F]
    W8 = np.ascontiguousarray(
        Wtail.reshape(2, 128, NT, 2, 256).transpose(2, 3, 1, 0, 4)
    )  # [NT, 2, 128, 2, 256]

    in_maps = []
    for c in range(NCORES):
        xs = x[c * TPC : (c + 1) * TPC]  # [TPC, IN_F]
        xTc = np.ascontiguousarray(
            xs[:, :KBF].T.reshape(KB, 128, TPC).transpose(1, 0, 2)
        ).astype(ml_dtypes.bfloat16)  # [128, KB, TPC]
        # x8[p, m, i, t] = xs[m*128 + t, KBF + i*128 + p] / 8
        xt8 = (xs[:, KBF:] * 0.125).astype(ml_dtypes.float8_e4m3)  # [TPC, 256]
        x8c = np.ascontiguousarray(
            xt8.reshape(MT, 128, 2, 128).transpose(3, 0, 2, 1)
        )  # [128, MT, 2, 128]
        in_maps.append({"xT": xTc, "x8": x8c, "W": W4, "W8": W8})
    return in_maps, bias


def _run(x, values, bias, col_indices, trace=False):
    from concourse.bass_utils import run_bass_kernel_spmd

    nc = _build_program()
    in_maps, bias_np = _prep_inputs(x, values, bias, col_indices)
    kwargs = {}
    if trace:
        import tempfile

        kwargs["tmpdir"] = tempfile.mkdtemp(prefix="bass_trace_")
    try:
        res = run_bass_kernel_spmd(
            nc, in_maps, list(range(NCORES)), trace=trace, **kwargs
        )
    except Exception:
        import time

        time.sleep(20)
        res = run_bass_kernel_spmd(
            nc, in_maps, list(range(NCORES)), trace=trace, **kwargs
        )
    out = np.concatenate([res.results[c]["out"] for c in range(NCORES)], axis=0)
    if np.any(bias_np):
        out = out + bias_np[None, :]
    return out, res


def kernel(x, values, bias, col_indices):
    out, _ = _run(x, values, bias, col_indices)
    return out
